# revision 38
# baseline (speedup 1.0000x reference)
"""Trainium2 Bass kernel for AutoRegressiveAdaptiveSpectralConv2d.

reference:  f = fft2(x)[..., :32, :32]
            o = einsum('btixy,tioxy->btoxy', f, R_w) * Ws_w
            o = (o * Wt/sum(Wt)).sum(t)            -> [B,1,U,32,32]
            out = ifft2(o, s=(256,256))            -> [B,1,U,256,256] complex64

v3 structure (8 cores, single SPMD launch, fp16 data / fp32 PSUM):
  Phase 1 runs in 3 waves of one (b,t) pair (32 images) per core:
    step A: PQT[w, (P|Q,kx)] = x-block(stationary) @ ats   (4 mm/img, 64 cols)
    step B: f_ps[(c|s,ky), (img,m)] = B2(stationary) @ PQT (2 mm per 8 imgs)
    combine on gpsimd: f_re = cP + sQ, f_im = cQ - sP -> f_stage fp16
    per-wave AllToAll redistributes f (b,t)-shard -> kx-shard, overlapped
    with the next wave's compute.
  Phase 2 (xy-sharded channel mix) is PSUM-accumulated across the 3 waves:
    wave j supplies k-chunk j (t = {j, 3+j} x i x re/im = 128 rows).
  One small AllToAll moves o to (b,o)-sharding; phase 3 does the
  zero-padded iFFT with 2-channel-batched 512-col matmuls.
Weights pre-scaled by 2^22 on host; host divides output by 2^38.
"""
import sys
import numpy as np

sys.path.insert(0, "/opt/trn_rl_repo")

import concourse.bass as bass
import concourse.bacc as bacc
import concourse.mybir as mybir
import concourse.tile as tile
from concourse import bass_utils

B, T, U, H, W = 4, 6, 32, 256, 256
MX, MY = 32, 32
NC = 8
PAIRS_PER_CORE = (B * T) // NC          # 3 (= waves)
CH_PER_CORE = (B * U) // NC             # 16
XY_PER_CORE = (MX * MY) // NC           # 128
W_SCALE = float(2 ** 22)
OUT_DESCALE = float(2 ** 22) * float(H * W)

F16 = mybir.dt.float16
F32 = mybir.dt.float32

# Cross-partition-base operands are illegal (NCC_IBIR297), and only one
# input may read PSUM (NCC_IBVF027): use per-cs PSUM tiles at base 0 and
# evict the sin products to SBUF before the combines.
CROSS_BASE = False


def _ap(t, offset, dims):
    """AP on a pool tile with explicit [step, count] dims (tile-relative)."""
    return bass.AP(t[:].tensor, offset, dims)


def build_nc(timing=False, local_exchange=False):
    nc = bacc.Bacc("TRN2", target_bir_lowering=False, debug=False, num_devices=NC)

    def ext_in(name, shape, dt):
        if timing:
            return nc.dram_tensor(name, shape, dt)
        return nc.dram_tensor(name, shape, dt, kind="ExternalInput")

    xsh = ext_in("xsh", [PAIRS_PER_CORE, U, H, W], F16)
    W2d = ext_in("W2d", [3, 4, 128, 2048], F16)
    ATs = ext_in("ATs", [H, 64], F16)
    B2T = ext_in("B2T", [W, 64], F16)
    IDN = ext_in("IDN", [128, 128], F16)
    CsT = ext_in("CsT", [MX, 512], F16)
    CT2r = ext_in("CT2r", [128, 256], F16)
    CT2i = ext_in("CT2i", [128, 256], F16)
    # outp layout: [chl, hc, p, comp, w] with h = hc*128 + p
    if timing:
        outp = nc.dram_tensor("outp", [CH_PER_CORE, 2, 128, 2, 256], F16)
        dummy_in = nc.dram_tensor("dummy_in", [1, 64], F16, kind="ExternalInput")
        dummy_out = nc.dram_tensor("dummy_out", [1, 64], F16, kind="ExternalOutput")
    else:
        outp = nc.dram_tensor("outp", [CH_PER_CORE, 2, 128, 2, 256], F16,
                              kind="ExternalOutput")

    with tile.TileContext(nc) as tc:
        with (
            tc.tile_pool(name="dram", bufs=1, space="DRAM") as dram,
            tc.tile_pool(name="consts", bufs=1) as consts,
            tc.tile_pool(name="p1x", bufs=6) as p1x,
            tc.tile_pool(name="p1pq", bufs=2) as p1pq,
            tc.tile_pool(name="p1fs", bufs=2) as p1fs,
            tc.tile_pool(name="p2w", bufs=1) as p2w,
            tc.tile_pool(name="p2fk", bufs=2) as p2fk,
            tc.tile_pool(name="p2o", bufs=1) as p2osb,
            tc.tile_pool(name="p3sb", bufs=2) as p3sb,
        ):
            # DRAM exchange buffers: chunk layout per dst/src = [xy128, (i,c)64]
            f_in = [dram.tile([NC, XY_PER_CORE, 64], F16, tag=f"fin{j}",
                              name=f"f_in{j}") for j in range(3)]
            f_out = [dram.tile([NC, XY_PER_CORE, 64], F16, tag=f"fout{j}",
                               name=f"f_out{j}") for j in range(3)]
            o_in = dram.tile([NC, 32, XY_PER_CORE], F16)
            o_out = dram.tile([NC, 32, XY_PER_CORE], F16)

            ats = consts.tile([128, 128], F16)   # [h%128, (hc, m=(P|Q,kx))]
            b2 = consts.tile([128, 128], F16)    # [w%128, (wc, (c|s, ky))]
            idn = consts.tile([128, 128], F16)
            cst = consts.tile([MX, 512], F16)
            ct2r = consts.tile([128, 256], F16)
            ct2i = consts.tile([128, 256], F16)

            # SBUF consts (scalar queue, emitted first)
            nc.scalar.dma_start(
                _ap(ats, 0, [[128, 128], [64, 2], [1, 64]]),
                bass.AP(ATs, 0, [[64, 128], [8192, 2], [1, 64]]))
            nc.scalar.dma_start(
                _ap(b2, 0, [[128, 128], [64, 2], [1, 64]]),
                bass.AP(B2T, 0, [[64, 128], [8192, 2], [1, 64]]))
            nc.scalar.dma_start(idn[:], IDN[:])
            nc.scalar.dma_start(cst[:], CsT[:])
            nc.scalar.dma_start(ct2r[:], CT2r[:])
            nc.scalar.dma_start(ct2i[:], CT2i[:])

            w_all = p2w.tile([128, 3 * 8192], F16, tag="wsb")

            # phase-2 accumulator (lives across all 3 waves)
            p2ctx = tc.tile_pool(name="p2ps", bufs=1, space="PSUM")
            p2ps = p2ctx.__enter__()
            p1actx = tc.tile_pool(name="p1psa", bufs=2, space="PSUM")
            p1psa = p1actx.__enter__()
            p1bctx = tc.tile_pool(name="p1psb", bufs=2, space="PSUM")
            p1psb = p1bctx.__enter__()
            # full-bank tile; partitions 0-63 used. Explicitly zeroed, then
            # all phase-2 matmuls pure-accumulate (order-independent).
            o_ps = p2ps.tile([128, 512], F32, tag="ops")  # [(oh,c',o), (b,kyy,kxl)]
            nc.vector.memset(_ap(o_ps, 0, [[512, 64], [1, 512]]), 0.0)

            cpeng = [nc.scalar.copy, nc.vector.tensor_copy]
            fk_tiles = []

            # ---- all x loads up-front on sync so no later-waiting DMA
            # (fk_raw waits on a collective) head-of-line blocks them ----
            x_tiles = []
            for j in range(3):
                for half in range(2):
                    x_sb = p1x.tile([128, 8192], F16, tag="x")
                    # wave 2 rides scalar (idle until copies ramp ~25us)
                    eng = nc.scalar if j == 2 else nc.sync
                    eng.dma_start(
                        _ap(x_sb, 0, [[8192, 128], [512, 16], [256, 2], [1, 256]]),
                        bass.AP(xsh, (j * U + half * 16) * H * W,
                                [[256, 128], [65536, 16], [32768, 2], [1, 256]]))
                    x_tiles.append(x_sb)
            # W2 after x on the same queue: [128 krow, (j3, kxl4, kyy32, m64)]
            for j in range(3):
                nc.sync.dma_start(
                    _ap(w_all, j * 8192, [[24576, 128], [2048, 4], [1, 2048]]),
                    bass.AP(W2d, j * 4 * 128 * 2048,
                            [[2048, 128], [128 * 2048, 4], [1, 2048]]))

            def wave(j):
                x_halves = x_tiles[2 * j:2 * j + 2]
                # ---- step A: PQT[w, m] per image, 4-img PSUM banks ----
                pqt_sb = p1pq.tile([128, 4096], F16, tag="pqt_sb")
                nevict = 0
                for half in range(2):
                    x_sb = x_halves[half]
                    for bk in range(4):
                        pqt_ps = p1psa.tile([128, 512], F32, tag="pqt_ps")
                        for il in range(4):
                            for wc in range(2):
                                for hc in range(2):
                                    nc.tensor.matmul(
                                        _ap(pqt_ps, (il * 2 + wc) * 64,
                                            [[512, 128], [1, 64]]),
                                        _ap(x_sb, (bk * 4 + il) * 512 + hc * 256
                                            + wc * 128, [[8192, 128], [1, 128]]),
                                        _ap(ats, hc * 64, [[128, 128], [1, 64]]),
                                        start=(hc == 0), stop=(hc == 1))
                        cpeng[nevict % 2](
                            _ap(pqt_sb, (half * 16 + bk * 4) * 128,
                                [[4096, 128], [1, 512]]),
                            pqt_ps[:])
                        nevict += 1
                # ---- step B + combine: f_stage [ky, (kx, i, c)] fp16 ----
                f_stage = p1fs.tile([32, 2048], F16, tag="fstage")
                for grp in range(4):       # 8 images each
                    mov = lambda wc, moff, cnt=64: _ap(
                        pqt_sb, grp * 8 * 128 + wc * 64 + moff,
                        [[4096, 128], [128, 8], [1, cnt]])
                    if CROSS_BASE:
                        f_ps = p1psb.tile([128, 512], F32, tag="fps")
                        for wc in range(2):
                            nc.tensor.matmul(
                                _ap(f_ps, 0, [[512, 64], [1, 512]]),
                                _ap(b2, wc * 64, [[128, 128], [1, 64]]),
                                mov(wc, 0),
                                start=(wc == 0), stop=(wc == 1))
                        # evict whole [64,512] to SBUF; combines read SBUF
                        f_ev = p1fs.tile([64, 512], F16, tag="fev")
                        cpeng[grp % 2](f_ev[:], _ap(f_ps, 0,
                                                    [[512, 64], [1, 512]]))
                        srcs = {  # (partition base, col base)
                            "cP": _ap(f_ev, 0, [[512, 32], [64, 8], [1, 32]]),
                            "sQ": _ap(f_ev, 32 * 512 + 32,
                                      [[512, 32], [64, 8], [1, 32]]),
                            "cQ": _ap(f_ev, 32, [[512, 32], [64, 8], [1, 32]]),
                            "sP": _ap(f_ev, 32 * 512,
                                      [[512, 32], [64, 8], [1, 32]]),
                        }
                    else:
                        ps_c = p1psb.tile([128, 512], F32, tag="fps_c")
                        ps_s = p1psb.tile([128, 512], F32, tag="fps_s")
                        for wc in range(2):
                            for cs, pst in ((0, ps_c), (1, ps_s)):
                                nc.tensor.matmul(
                                    _ap(pst, 0, [[512, 32], [1, 512]]),
                                    _ap(b2, wc * 64 + cs * 32,
                                        [[128, 128], [1, 32]]),
                                    mov(wc, 0),
                                    start=(wc == 0), stop=(wc == 1))
                        # evict sin products to SBUF (aligned partitions)
                        f_ev = p1fs.tile([32, 512], F16, tag="fev")
                        cpeng[grp % 2](f_ev[:], _ap(ps_s, 0,
                                                    [[512, 32], [1, 512]]))
                        srcs = {
                            "cP": _ap(ps_c, 0, [[512, 32], [64, 8], [1, 32]]),
                            "sQ": _ap(f_ev, 32, [[512, 32], [64, 8], [1, 32]]),
                            "cQ": _ap(ps_c, 32, [[512, 32], [64, 8], [1, 32]]),
                            "sP": _ap(f_ev, 0, [[512, 32], [64, 8], [1, 32]]),
                        }
                    # f_re = cP + sQ ; f_im = cQ - sP  (dst col = kx*64+i*2+c)
                    nc.vector.scalar_tensor_tensor(
                        _ap(f_stage, grp * 16, [[2048, 32], [2, 8], [64, 32]]),
                        srcs["cP"], 1.0, srcs["sQ"],
                        mybir.AluOpType.mult, mybir.AluOpType.add)
                    nc.vector.scalar_tensor_tensor(
                        _ap(f_stage, grp * 16 + 1,
                            [[2048, 32], [2, 8], [64, 32]]),
                        srcs["cQ"], 1.0, srcs["sP"],
                        mybir.AluOpType.mult, mybir.AluOpType.subtract)
                # ---- store to f_in[j]: chunk[d] = [xy128, (i,c)64] ----
                nc.gpsimd.dma_start(
                    bass.AP(f_in[j][:].tensor, 0,
                            [[64, 32], [8192, 8], [2048, 4], [1, 64]]),
                    _ap(f_stage, 0, [[2048, 32], [256, 8], [64, 4], [1, 64]]))
                # ---- AllToAll wave j ----
                if local_exchange:
                    nc.sync.dma_start(f_out[j][:], f_in[j][:])
                else:
                    nc.gpsimd.collective_compute(
                        "AllToAll", mybir.AluOpType.bypass,
                        replica_groups=[list(range(NC))],
                        ins=[f_in[j].opt()], outs=[f_out[j].opt()])
                # ---- fk load: [128 krow=(tt,i,c), (b4, xy128)] ----
                # straight load [xy, (b,tt,ic)] then 4 XBAR transposes
                # -> fk [128 = (tt,i,c), (b4, xy128)]
                fk_raw = p2fk.tile([128, 512], F16, tag="fk_raw")
                nc.sync.dma_start(
                    _ap(fk_raw, 0, [[512, 128], [64, 8], [1, 64]]),
                    bass.AP(f_out[j][:].tensor, 0,
                            [[64, 128], [8192, 8], [1, 64]]))
                fk = p2fk.tile([128, 512], F16, tag="fk")
                for bp in range(2):
                    tr_ps = p1psa.tile([128, 256], F16, tag="tr", bufs=1)
                    for bh in range(2):
                        b = bp * 2 + bh
                        nc.tensor.transpose(
                            _ap(tr_ps, bh * 128, [[256, 128], [1, 128]]),
                            _ap(fk_raw, b * 128, [[512, 128], [1, 128]]),
                            idn[:])
                    cpeng[bp % 2](_ap(fk, bp * 256, [[512, 128], [1, 256]]),
                                  tr_ps[:])
                fk_tiles.append(fk)

            def p2_chunk(j):
                # o_ps cols = (b4, kyy32, kxl4)
                fk = fk_tiles[j]
                for kxl in range(4):
                    for kyy in range(32):
                        nc.tensor.matmul(
                            _ap(o_ps, kyy * 4 + kxl, [[512, 64], [128, 4]]),
                            _ap(w_all, j * 8192 + kxl * 2048 + kyy * 64,
                                [[24576, 128], [1, 64]]),
                            _ap(fk, kxl * 32 + kyy, [[512, 128], [128, 4]]),
                            start=False, stop=False, skip_group_check=True)

            # ---- schedule: w0, w1, p2c0, w2, p2c1, p2c2 ----
            wave(0)
            wave(1)
            p2_chunk(0)
            wave(2)
            p2_chunk(1)
            p2_chunk(2)

            # ---- o evict + exchange ----
            # chunk layout per dst: [32 ky, 128 = (c,o_lo)*4 + kxl]
            o_sb = p2osb.tile([64, 512], F16, tag="osb")
            nc.vector.tensor_copy(o_sb[:], _ap(o_ps, 0, [[512, 64], [1, 512]]))
            for d in range(NC):
                eng = (nc.sync, nc.scalar, nc.gpsimd)[d % 3]
                eng.dma_start(
                    bass.AP(o_in[:].tensor, d * 4096,
                            [[4, 32], [128, 32], [1, 4]]),
                    _ap(o_sb, (d % 2) * 32 * 512 + (d // 2) * 128,
                        [[512, 32], [4, 32], [1, 4]]))
            p1bctx.__exit__(None, None, None)
            p1actx.__exit__(None, None, None)
            p2ctx.__exit__(None, None, None)
            if local_exchange:
                nc.sync.dma_start(o_out[:], o_in[:])
            else:
                nc.gpsimd.collective_compute(
                    "AllToAll", mybir.AluOpType.bypass,
                    replica_groups=[list(range(NC))],
                    ins=[o_in.opt()], outs=[o_out.opt()])

            # ---- phase 3: zero-padded iFFT, 2 channels per group ----
            # straight load then same-partition col rearrange
            ot_raw = consts.tile([MX, CH_PER_CORE * 64], F16)  # [ky,(s,p,kxl)]
            nc.sync.dma_start(
                _ap(ot_raw, 0, [[1024, 32], [128, 8], [1, 128]]),
                bass.AP(o_out[:].tensor, 0, [[128, 32], [4096, 8], [1, 128]]))
            ot = consts.tile([MX, CH_PER_CORE * 64], F16)  # [ky,(ch,c,kx)]
            for s in range(NC):
                eng = (nc.vector.tensor_copy, nc.gpsimd.tensor_copy,
                       nc.scalar.copy)[s % 3]
                eng(_ap(ot, s * 4, [[1024, 32], [64, 16], [32, 2], [1, 4]]),
                    _ap(ot_raw, s * 128, [[1024, 32], [4, 16], [64, 2], [1, 4]]))
            p3actx = tc.tile_pool(name="p3psa", bufs=2, space="PSUM")
            p3psa = p3actx.__enter__()
            p3bctx = tc.tile_pool(name="p3psb", bufs=4, space="PSUM")
            p3psb = p3bctx.__enter__()
            def g_group(gi):
                g2 = p3sb.tile([128, 512], F16, tag="g2")
                for cl in range(2):
                    chl = gi * 2 + cl
                    g_ps = p3psa.tile([128, 256], F32, tag="g_ps")
                    for cs in range(2):
                        nc.tensor.matmul(
                            _ap(g_ps, cs * 64 * 256, [[256, 64], [1, 256]]),
                            _ap(ot, chl * 64, [[1024, 32], [1, 64]]),
                            _ap(cst, cs * 256, [[512, 32], [1, 256]]),
                            start=True, stop=True)
                    cpeng[cl](_ap(g2, cl * 256, [[512, 128], [1, 256]]),
                              g_ps[:])
                return g2

            def p_group(gi, g2):
                out_int = p3sb.tile([128, 2048], F16, tag="oint")
                ev = 0
                for hc in range(2):
                    for comp in range(2):
                        p_ps = p3psb.tile([128, 512], F32, tag="p_ps")
                        nc.tensor.matmul(
                            _ap(p_ps, 0, [[512, 128], [1, 512]]),
                            _ap(ct2r if comp == 0 else ct2i, hc * 128,
                                [[256, 128], [1, 128]]),
                            g2[:], start=True, stop=True)
                        eng = (nc.scalar.copy, nc.vector.tensor_copy)[ev % 2]
                        eng(_ap(out_int, hc * 512 + comp * 256,
                                [[2048, 128], [1024, 2], [1, 256]]),
                            _ap(p_ps, 0, [[512, 128], [256, 2], [1, 256]]))
                        ev += 1
                for cl in range(2):
                    chl = gi * 2 + cl
                    # keep scalar free for PSUM evicts in phase 3
                    eng = (nc.sync, nc.gpsimd)[chl % 2]
                    eng.dma_start(
                        bass.AP(outp, chl * 2 * H * W,
                                [[512, 128], [65536, 2], [1, 512]]),
                        _ap(out_int, cl * 1024,
                            [[2048, 128], [512, 2], [1, 512]]))

            # software pipeline: g one group ahead so the PE never waits
            # on the g-evict copies before each p-step
            prev = None
            for gi in range(CH_PER_CORE // 2):
                g2 = g_group(gi)
                if prev is not None:
                    p_group(*prev)
                prev = (gi, g2)
            p_group(*prev)
            p3bctx.__exit__(None, None, None)
            p3actx.__exit__(None, None, None)
            if timing:
                nc.sync.dma_start(bass.AP(dummy_out, 0, [[64, 1], [1, 64]]),
                                  bass.AP(dummy_in, 0, [[64, 1], [1, 64]]))
    nc.compile()
    return nc


_NC_CACHE = None


def _get_nc():
    global _NC_CACHE
    if _NC_CACHE is None:
        _NC_CACHE = build_nc()
    return _NC_CACHE


def _host_prep(x, R_w, Ws_w, Wt_w):
    x = np.asarray(x)
    R_w = np.asarray(R_w)
    Ws_w = np.asarray(Ws_w, dtype=np.float32)
    Wt_w = np.asarray(Wt_w, dtype=np.float32)

    xf = x.reshape(B * T, U, H, W).astype(np.float16)

    h = np.arange(H)[:, None]
    k = np.arange(MX)[None, :]
    ang = 2.0 * np.pi * h * k / H
    ATs = np.concatenate([np.cos(ang), -np.sin(ang)], axis=1).astype(np.float16)

    w = np.arange(W)[:, None]
    ky = np.arange(MY)[None, :]
    angb = 2.0 * np.pi * w * ky / W
    B2T = np.concatenate([np.cos(angb), np.sin(angb)], axis=1).astype(np.float16)

    xg = np.arange(MX)[:, None]
    wg = np.arange(W)[None, :]
    ang2 = 2.0 * np.pi * xg * wg / W
    cos2 = np.cos(ang2).astype(np.float32)
    sin2 = np.sin(ang2).astype(np.float32)
    CsT = np.concatenate([cos2, sin2], axis=1).astype(np.float16)
    CT2r = np.concatenate([cos2, -sin2, -sin2, -cos2], axis=0).astype(np.float16)
    CT2i = np.concatenate([sin2, cos2, cos2, -sin2], axis=0).astype(np.float16)

    wt = (Wt_w / Wt_w.sum()).reshape(T)
    Wc = (R_w * Ws_w[None, None, None]
          * wt[:, None, None, None, None].astype(np.float32) * W_SCALE)
    Wr = np.real(Wc).astype(np.float32)
    Wi = np.imag(Wc).astype(np.float32)
    # [xy, t, i, o]
    Wr_x = Wr.transpose(3, 4, 0, 1, 2).reshape(MX * MY, T, U, U)
    Wi_x = Wi.transpose(3, 4, 0, 1, 2).reshape(MX * MY, T, U, U)
    # arr[xy, j, tt, i, c, m=(oh,c',o_lo)]
    arr = np.empty((MX * MY, 3, 2, U, 2, 2, 2, 16), np.float16)
    for j in range(3):
        for tt in range(2):
            t = j + 3 * tt
            wr = Wr_x[:, t].reshape(MX * MY, U, 2, 16)
            wi = Wi_x[:, t].reshape(MX * MY, U, 2, 16)
            arr[:, j, tt, :, 0, :, 0, :] = wr
            arr[:, j, tt, :, 0, :, 1, :] = wi
            arr[:, j, tt, :, 1, :, 0, :] = -wi
            arr[:, j, tt, :, 1, :, 1, :] = wr

    in_maps = []
    for c in range(NC):
        w2c = (arr[c * XY_PER_CORE:(c + 1) * XY_PER_CORE]
               .reshape(4, 32, 3, 128, 64).transpose(2, 0, 3, 1, 4)
               .reshape(3, 4, 128, 2048))
        in_maps.append({
            "xsh": np.ascontiguousarray(xf[c * PAIRS_PER_CORE:(c + 1) * PAIRS_PER_CORE]),
            "W2d": np.ascontiguousarray(w2c),
            "ATs": ATs,
            "B2T": B2T,
            "IDN": np.eye(128, dtype=np.float16),
            "CsT": CsT,
            "CT2r": CT2r,
            "CT2i": CT2i,
        })
    return in_maps


def _host_post(results):
    out = np.empty((B, 1, U, H, W), np.complex64)
    inv = np.float32(1.0 / OUT_DESCALE)
    for c in range(NC):
        # raw layout [chl, hc, p, comp, w]; h = hc*128 + p
        arr = (np.asarray(results[c]["outp"])
               .reshape(CH_PER_CORE, 2, 128, 2, 256).astype(np.float32))
        arr = arr.transpose(0, 3, 1, 2, 4).reshape(CH_PER_CORE, 2, H, W)
        carr = (arr[:, 0] + 1j * arr[:, 1]).astype(np.complex64)
        for j in range(CH_PER_CORE):
            ch = c * CH_PER_CORE + j
            out[ch // U, 0, ch % U] = carr[j] * inv
    return out


def kernel(**inputs):
    nc = _get_nc()
    in_maps = _host_prep(inputs["input"], inputs["R_w"], inputs["Ws_w"], inputs["Wt_w"])
    res = bass_utils.run_bass_kernel_spmd(nc, in_maps, core_ids=list(range(NC)))
    return _host_post(res.results)


# revision 40
# speedup vs baseline: 1.0296x; 1.0296x over previous
"""Trainium2 Bass kernel for AutoRegressiveAdaptiveSpectralConv2d.

reference:  f = fft2(x)[..., :32, :32]
            o = einsum('btixy,tioxy->btoxy', f, R_w) * Ws_w
            o = (o * Wt/sum(Wt)).sum(t)            -> [B,1,U,32,32]
            out = ifft2(o, s=(256,256))            -> [B,1,U,256,256] complex64

v3 structure (8 cores, single SPMD launch, fp16 data / fp32 PSUM):
  Phase 1 runs in 3 waves of one (b,t) pair (32 images) per core:
    step A: PQT[w, (P|Q,kx)] = x-block(stationary) @ ats   (4 mm/img, 64 cols)
    step B: f_ps[(c|s,ky), (img,m)] = B2(stationary) @ PQT (2 mm per 8 imgs)
    combine on gpsimd: f_re = cP + sQ, f_im = cQ - sP -> f_stage fp16
    per-wave AllToAll redistributes f (b,t)-shard -> kx-shard, overlapped
    with the next wave's compute.
  Phase 2 (xy-sharded channel mix) is PSUM-accumulated across the 3 waves:
    wave j supplies k-chunk j (t = {j, 3+j} x i x re/im = 128 rows).
  One small AllToAll moves o to (b,o)-sharding; phase 3 does the
  zero-padded iFFT with 2-channel-batched 512-col matmuls.
Weights pre-scaled by 2^22 on host; host divides output by 2^38.
"""
import sys
import numpy as np

sys.path.insert(0, "/opt/trn_rl_repo")

import concourse.bass as bass
import concourse.bacc as bacc
import concourse.mybir as mybir
import concourse.tile as tile
from concourse import bass_utils

B, T, U, H, W = 4, 6, 32, 256, 256
MX, MY = 32, 32
NC = 8
PAIRS_PER_CORE = (B * T) // NC          # 3 (= waves)
CH_PER_CORE = (B * U) // NC             # 16
XY_PER_CORE = (MX * MY) // NC           # 128
W_SCALE = float(2 ** 22)
OUT_DESCALE = float(2 ** 22) * float(H * W)

F16 = mybir.dt.float16
F32 = mybir.dt.float32

# Cross-partition-base operands are illegal (NCC_IBIR297), and only one
# input may read PSUM (NCC_IBVF027): use per-cs PSUM tiles at base 0 and
# evict the sin products to SBUF before the combines.
CROSS_BASE = False


def _ap(t, offset, dims):
    """AP on a pool tile with explicit [step, count] dims (tile-relative)."""
    return bass.AP(t[:].tensor, offset, dims)


def build_nc(timing=False, local_exchange=False):
    nc = bacc.Bacc("TRN2", target_bir_lowering=False, debug=False, num_devices=NC)

    def ext_in(name, shape, dt):
        if timing:
            return nc.dram_tensor(name, shape, dt)
        return nc.dram_tensor(name, shape, dt, kind="ExternalInput")

    xsh = ext_in("xsh", [PAIRS_PER_CORE, U, H, W], F16)
    W2d = ext_in("W2d", [3, 4, 128, 2048], F16)
    ATs = ext_in("ATs", [H, 64], F16)
    B2T = ext_in("B2T", [W, 64], F16)
    IDN = ext_in("IDN", [128, 128], F16)
    CsT = ext_in("CsT", [MX, 512], F16)
    CT2r = ext_in("CT2r", [128, 256], F16)
    CT2i = ext_in("CT2i", [128, 256], F16)
    # outp layout: [chl, hc, p, comp, w] with h = hc*128 + p
    if timing:
        outp = nc.dram_tensor("outp", [CH_PER_CORE, 2, 128, 2, 256], F16)
        dummy_in = nc.dram_tensor("dummy_in", [1, 64], F16, kind="ExternalInput")
        dummy_out = nc.dram_tensor("dummy_out", [1, 64], F16, kind="ExternalOutput")
    else:
        outp = nc.dram_tensor("outp", [CH_PER_CORE, 2, 128, 2, 256], F16,
                              kind="ExternalOutput")

    with tile.TileContext(nc) as tc:
        with (
            tc.tile_pool(name="dram", bufs=1, space="DRAM") as dram,
            tc.tile_pool(name="consts", bufs=1) as consts,
            tc.tile_pool(name="p1x", bufs=6) as p1x,
            tc.tile_pool(name="p1pq", bufs=2) as p1pq,
            tc.tile_pool(name="p1fs", bufs=2) as p1fs,
            tc.tile_pool(name="p2w", bufs=1) as p2w,
            tc.tile_pool(name="p2fk", bufs=2) as p2fk,
            tc.tile_pool(name="p2o", bufs=1) as p2osb,
            tc.tile_pool(name="p3sb", bufs=2) as p3sb,
        ):
            # DRAM exchange buffers: chunk layout per dst/src = [xy128, (i,c)64]
            f_in = [dram.tile([NC, XY_PER_CORE, 64], F16, tag=f"fin{j}",
                              name=f"f_in{j}") for j in range(3)]
            f_out = [dram.tile([NC, XY_PER_CORE, 64], F16, tag=f"fout{j}",
                               name=f"f_out{j}") for j in range(3)]
            o_in = dram.tile([NC, 32, XY_PER_CORE], F16)
            o_out = dram.tile([NC, 32, XY_PER_CORE], F16)

            ats = consts.tile([128, 128], F16)   # [h%128, (hc, m=(P|Q,kx))]
            b2 = consts.tile([128, 128], F16)    # [w%128, (wc, (c|s, ky))]
            idn = consts.tile([128, 128], F16)
            cst = consts.tile([MX, 512], F16)
            ct2r = consts.tile([128, 256], F16)
            ct2i = consts.tile([128, 256], F16)

            # SBUF consts (scalar queue, emitted first)
            nc.scalar.dma_start(
                _ap(ats, 0, [[128, 128], [64, 2], [1, 64]]),
                bass.AP(ATs, 0, [[64, 128], [8192, 2], [1, 64]]))
            nc.scalar.dma_start(
                _ap(b2, 0, [[128, 128], [64, 2], [1, 64]]),
                bass.AP(B2T, 0, [[64, 128], [8192, 2], [1, 64]]))
            nc.scalar.dma_start(idn[:], IDN[:])
            nc.scalar.dma_start(cst[:], CsT[:])
            nc.scalar.dma_start(ct2r[:], CT2r[:])
            nc.scalar.dma_start(ct2i[:], CT2i[:])

            w_all = p2w.tile([128, 3 * 8192], F16, tag="wsb")

            # phase-2 accumulator (lives across all 3 waves)
            p2ctx = tc.tile_pool(name="p2ps", bufs=1, space="PSUM")
            p2ps = p2ctx.__enter__()
            p1actx = tc.tile_pool(name="p1psa", bufs=2, space="PSUM")
            p1psa = p1actx.__enter__()
            p1bctx = tc.tile_pool(name="p1psb", bufs=2, space="PSUM")
            p1psb = p1bctx.__enter__()
            # full-bank tile; partitions 0-63 used. Explicitly zeroed, then
            # all phase-2 matmuls pure-accumulate (order-independent).
            o_ps = p2ps.tile([128, 512], F32, tag="ops")  # [(oh,c',o), (b,kyy,kxl)]
            nc.vector.memset(_ap(o_ps, 0, [[512, 64], [1, 512]]), 0.0)

            cpeng = [nc.scalar.copy, nc.vector.tensor_copy]
            fk_tiles = []

            # ---- all x loads up-front on sync so no later-waiting DMA
            # (fk_raw waits on a collective) head-of-line blocks them ----
            x_tiles = []
            for j in range(3):
                for half in range(2):
                    x_sb = p1x.tile([128, 8192], F16, tag="x")
                    # wave 2 rides scalar (idle until copies ramp ~25us)
                    eng = nc.scalar if j == 2 else nc.sync
                    eng.dma_start(
                        _ap(x_sb, 0, [[8192, 128], [512, 16], [256, 2], [1, 256]]),
                        bass.AP(xsh, (j * U + half * 16) * H * W,
                                [[256, 128], [65536, 16], [32768, 2], [1, 256]]))
                    x_tiles.append(x_sb)
            # W2 after x on the same queue: [128 krow, (j3, kxl4, kyy32, m64)]
            for j in range(3):
                nc.sync.dma_start(
                    _ap(w_all, j * 8192, [[24576, 128], [2048, 4], [1, 2048]]),
                    bass.AP(W2d, j * 4 * 128 * 2048,
                            [[2048, 128], [128 * 2048, 4], [1, 2048]]))

            def wave(j):
                x_halves = x_tiles[2 * j:2 * j + 2]
                # ---- step A: PQT[w, m] per image, 4-img PSUM banks ----
                pqt_sb = p1pq.tile([128, 4096], F16, tag="pqt_sb")
                nevict = 0
                for half in range(2):
                    x_sb = x_halves[half]
                    for bk in range(4):
                        pqt_ps = p1psa.tile([128, 512], F32, tag="pqt_ps")
                        for il in range(4):
                            for wc in range(2):
                                for hc in range(2):
                                    nc.tensor.matmul(
                                        _ap(pqt_ps, (il * 2 + wc) * 64,
                                            [[512, 128], [1, 64]]),
                                        _ap(x_sb, (bk * 4 + il) * 512 + hc * 256
                                            + wc * 128, [[8192, 128], [1, 128]]),
                                        _ap(ats, hc * 64, [[128, 128], [1, 64]]),
                                        start=(hc == 0), stop=(hc == 1))
                        cpeng[nevict % 2](
                            _ap(pqt_sb, (half * 16 + bk * 4) * 128,
                                [[4096, 128], [1, 512]]),
                            pqt_ps[:])
                        nevict += 1
                # ---- step B + combine: f_stage [ky, (kx, i, c)] fp16 ----
                f_stage = p1fs.tile([32, 2048], F16, tag="fstage")
                for grp in range(4):       # 8 images each
                    mov = lambda wc, moff, cnt=64: _ap(
                        pqt_sb, grp * 8 * 128 + wc * 64 + moff,
                        [[4096, 128], [128, 8], [1, cnt]])
                    if CROSS_BASE:
                        f_ps = p1psb.tile([128, 512], F32, tag="fps")
                        for wc in range(2):
                            nc.tensor.matmul(
                                _ap(f_ps, 0, [[512, 64], [1, 512]]),
                                _ap(b2, wc * 64, [[128, 128], [1, 64]]),
                                mov(wc, 0),
                                start=(wc == 0), stop=(wc == 1))
                        # evict whole [64,512] to SBUF; combines read SBUF
                        f_ev = p1fs.tile([64, 512], F16, tag="fev")
                        cpeng[grp % 2](f_ev[:], _ap(f_ps, 0,
                                                    [[512, 64], [1, 512]]))
                        srcs = {  # (partition base, col base)
                            "cP": _ap(f_ev, 0, [[512, 32], [64, 8], [1, 32]]),
                            "sQ": _ap(f_ev, 32 * 512 + 32,
                                      [[512, 32], [64, 8], [1, 32]]),
                            "cQ": _ap(f_ev, 32, [[512, 32], [64, 8], [1, 32]]),
                            "sP": _ap(f_ev, 32 * 512,
                                      [[512, 32], [64, 8], [1, 32]]),
                        }
                    else:
                        ps_c = p1psb.tile([128, 512], F32, tag="fps_c")
                        ps_s = p1psb.tile([128, 512], F32, tag="fps_s")
                        for wc in range(2):
                            for cs, pst in ((0, ps_c), (1, ps_s)):
                                nc.tensor.matmul(
                                    _ap(pst, 0, [[512, 32], [1, 512]]),
                                    _ap(b2, wc * 64 + cs * 32,
                                        [[128, 128], [1, 32]]),
                                    mov(wc, 0),
                                    start=(wc == 0), stop=(wc == 1))
                        # evict sin products to SBUF (aligned partitions)
                        f_ev = p1fs.tile([32, 512], F16, tag="fev")
                        cpeng[grp % 2](f_ev[:], _ap(ps_s, 0,
                                                    [[512, 32], [1, 512]]))
                        srcs = {
                            "cP": _ap(ps_c, 0, [[512, 32], [64, 8], [1, 32]]),
                            "sQ": _ap(f_ev, 32, [[512, 32], [64, 8], [1, 32]]),
                            "cQ": _ap(ps_c, 32, [[512, 32], [64, 8], [1, 32]]),
                            "sP": _ap(f_ev, 0, [[512, 32], [64, 8], [1, 32]]),
                        }
                    # f_re = cP + sQ ; f_im = cQ - sP  (dst col = kx*64+i*2+c)
                    nc.vector.scalar_tensor_tensor(
                        _ap(f_stage, grp * 16, [[2048, 32], [2, 8], [64, 32]]),
                        srcs["cP"], 1.0, srcs["sQ"],
                        mybir.AluOpType.mult, mybir.AluOpType.add)
                    nc.vector.scalar_tensor_tensor(
                        _ap(f_stage, grp * 16 + 1,
                            [[2048, 32], [2, 8], [64, 32]]),
                        srcs["cQ"], 1.0, srcs["sP"],
                        mybir.AluOpType.mult, mybir.AluOpType.subtract)
                # ---- store to f_in[j]: chunk[d] = [xy128, (i,c)64] ----
                nc.gpsimd.dma_start(
                    bass.AP(f_in[j][:].tensor, 0,
                            [[64, 32], [8192, 8], [2048, 4], [1, 64]]),
                    _ap(f_stage, 0, [[2048, 32], [256, 8], [64, 4], [1, 64]]))
                # ---- AllToAll wave j ----
                if local_exchange:
                    nc.sync.dma_start(f_out[j][:], f_in[j][:])
                else:
                    nc.gpsimd.collective_compute(
                        "AllToAll", mybir.AluOpType.bypass,
                        replica_groups=[list(range(NC))],
                        ins=[f_in[j].opt()], outs=[f_out[j].opt()])
                # ---- fk load: [128 krow=(tt,i,c), (b4, xy128)] ----
                # straight load [xy, (b,tt,ic)] then 4 XBAR transposes
                # -> fk [128 = (tt,i,c), (b4, xy128)]
                fk_raw = p2fk.tile([128, 512], F16, tag="fk_raw")
                nc.sync.dma_start(
                    _ap(fk_raw, 0, [[512, 128], [64, 8], [1, 64]]),
                    bass.AP(f_out[j][:].tensor, 0,
                            [[64, 128], [8192, 8], [1, 64]]))
                fk = p2fk.tile([128, 512], F16, tag="fk")
                for bp in range(2):
                    tr_ps = p1psa.tile([128, 256], F16, tag="tr", bufs=1)
                    for bh in range(2):
                        b = bp * 2 + bh
                        nc.tensor.transpose(
                            _ap(tr_ps, bh * 128, [[256, 128], [1, 128]]),
                            _ap(fk_raw, b * 128, [[512, 128], [1, 128]]),
                            idn[:])
                    cpeng[bp % 2](_ap(fk, bp * 256, [[512, 128], [1, 256]]),
                                  tr_ps[:])
                fk_tiles.append(fk)

            def p2_chunk(j):
                # o_ps cols = (b4, kyy32, kxl4)
                fk = fk_tiles[j]
                for kxl in range(4):
                    for kyy in range(32):
                        nc.tensor.matmul(
                            _ap(o_ps, kyy * 4 + kxl, [[512, 64], [128, 4]]),
                            _ap(w_all, j * 8192 + kxl * 2048 + kyy * 64,
                                [[24576, 128], [1, 64]]),
                            _ap(fk, kxl * 32 + kyy, [[512, 128], [128, 4]]),
                            start=False, stop=False, skip_group_check=True)

            # ---- schedule: w0, w1, p2c0, w2, p2c1, p2c2 ----
            wave(0)
            wave(1)
            p2_chunk(0)
            wave(2)
            p2_chunk(1)
            p2_chunk(2)

            # ---- o evict + exchange ----
            # chunk layout per dst: [32 ky, 128 = (c,o_lo)*4 + kxl]
            o_sb = p2osb.tile([64, 512], F16, tag="osb")
            nc.vector.tensor_copy(o_sb[:], _ap(o_ps, 0, [[512, 64], [1, 512]]))
            for d in range(NC):
                eng = (nc.sync, nc.scalar, nc.gpsimd)[d % 3]
                eng.dma_start(
                    bass.AP(o_in[:].tensor, d * 4096,
                            [[4, 32], [128, 32], [1, 4]]),
                    _ap(o_sb, (d % 2) * 32 * 512 + (d // 2) * 128,
                        [[512, 32], [4, 32], [1, 4]]))
            p1bctx.__exit__(None, None, None)
            p1actx.__exit__(None, None, None)
            p2ctx.__exit__(None, None, None)
            if local_exchange:
                nc.sync.dma_start(o_out[:], o_in[:])
            else:
                nc.gpsimd.collective_compute(
                    "AllToAll", mybir.AluOpType.bypass,
                    replica_groups=[list(range(NC))],
                    ins=[o_in.opt()], outs=[o_out.opt()])

            # ---- phase 3: zero-padded iFFT, 2 channels per group ----
            # straight load then same-partition col rearrange
            ot_raw = consts.tile([MX, CH_PER_CORE * 64], F16)  # [ky,(s,p,kxl)]
            nc.sync.dma_start(
                _ap(ot_raw, 0, [[1024, 32], [128, 8], [1, 128]]),
                bass.AP(o_out[:].tensor, 0, [[128, 32], [4096, 8], [1, 128]]))
            ot = consts.tile([MX, CH_PER_CORE * 64], F16)  # [ky,(ch,c,kx)]
            for s in range(NC):
                eng = (nc.vector.tensor_copy, nc.gpsimd.tensor_copy,
                       nc.scalar.copy)[s % 3]
                eng(_ap(ot, s * 4, [[1024, 32], [64, 16], [32, 2], [1, 4]]),
                    _ap(ot_raw, s * 128, [[1024, 32], [4, 16], [64, 2], [1, 4]]))
            p3actx = tc.tile_pool(name="p3psa", bufs=2, space="PSUM")
            p3psa = p3actx.__enter__()
            p3bctx = tc.tile_pool(name="p3psb", bufs=6, space="PSUM")
            p3psb = p3bctx.__enter__()
            def g_group(gi):
                g2 = p3sb.tile([128, 512], F16, tag="g2")
                for cl in range(2):
                    chl = gi * 2 + cl
                    g_ps = p3psa.tile([128, 256], F32, tag="g_ps")
                    for cs in range(2):
                        nc.tensor.matmul(
                            _ap(g_ps, cs * 64 * 256, [[256, 64], [1, 256]]),
                            _ap(ot, chl * 64, [[1024, 32], [1, 64]]),
                            _ap(cst, cs * 256, [[512, 32], [1, 256]]),
                            start=True, stop=True)
                    cpeng[cl](_ap(g2, cl * 256, [[512, 128], [1, 256]]),
                              g_ps[:])
                return g2

            def p_group(gi, g2):
                # per-hc staging so each store waits only 2 evicts
                ev = 0
                for hc in range(2):
                    oint = p3sb.tile([128, 1024], F16, tag=f"oint{hc}")
                    for comp in range(2):
                        p_ps = p3psb.tile([128, 512], F32, tag="p_ps")
                        nc.tensor.matmul(
                            _ap(p_ps, 0, [[512, 128], [1, 512]]),
                            _ap(ct2r if comp == 0 else ct2i, hc * 128,
                                [[256, 128], [1, 128]]),
                            g2[:], start=True, stop=True)
                        eng = (nc.scalar.copy, nc.vector.tensor_copy)[ev % 2]
                        eng(_ap(oint, comp * 256,
                                [[1024, 128], [512, 2], [1, 256]]),
                            _ap(p_ps, 0, [[512, 128], [256, 2], [1, 256]]))
                        ev += 1
                    for cl in range(2):
                        chl = gi * 2 + cl
                        eng = (nc.sync, nc.gpsimd)[(2 * gi + cl + hc) % 2]
                        eng.dma_start(
                            bass.AP(outp, chl * 2 * H * W + hc * 65536,
                                    [[512, 128], [1, 512]]),
                            _ap(oint, cl * 512, [[1024, 128], [1, 512]]))

            # software pipeline: g one group ahead so the PE never waits
            # on the g-evict copies before each p-step
            prev = None
            for gi in range(CH_PER_CORE // 2):
                g2 = g_group(gi)
                if prev is not None:
                    p_group(*prev)
                prev = (gi, g2)
            p_group(*prev)
            p3bctx.__exit__(None, None, None)
            p3actx.__exit__(None, None, None)
            if timing:
                nc.sync.dma_start(bass.AP(dummy_out, 0, [[64, 1], [1, 64]]),
                                  bass.AP(dummy_in, 0, [[64, 1], [1, 64]]))
    nc.compile()
    return nc


_NC_CACHE = None


def _get_nc():
    global _NC_CACHE
    if _NC_CACHE is None:
        _NC_CACHE = build_nc()
    return _NC_CACHE


def _host_prep(x, R_w, Ws_w, Wt_w):
    x = np.asarray(x)
    R_w = np.asarray(R_w)
    Ws_w = np.asarray(Ws_w, dtype=np.float32)
    Wt_w = np.asarray(Wt_w, dtype=np.float32)

    xf = x.reshape(B * T, U, H, W).astype(np.float16)

    h = np.arange(H)[:, None]
    k = np.arange(MX)[None, :]
    ang = 2.0 * np.pi * h * k / H
    ATs = np.concatenate([np.cos(ang), -np.sin(ang)], axis=1).astype(np.float16)

    w = np.arange(W)[:, None]
    ky = np.arange(MY)[None, :]
    angb = 2.0 * np.pi * w * ky / W
    B2T = np.concatenate([np.cos(angb), np.sin(angb)], axis=1).astype(np.float16)

    xg = np.arange(MX)[:, None]
    wg = np.arange(W)[None, :]
    ang2 = 2.0 * np.pi * xg * wg / W
    cos2 = np.cos(ang2).astype(np.float32)
    sin2 = np.sin(ang2).astype(np.float32)
    CsT = np.concatenate([cos2, sin2], axis=1).astype(np.float16)
    CT2r = np.concatenate([cos2, -sin2, -sin2, -cos2], axis=0).astype(np.float16)
    CT2i = np.concatenate([sin2, cos2, cos2, -sin2], axis=0).astype(np.float16)

    wt = (Wt_w / Wt_w.sum()).reshape(T)
    Wc = (R_w * Ws_w[None, None, None]
          * wt[:, None, None, None, None].astype(np.float32) * W_SCALE)
    Wr = np.real(Wc).astype(np.float32)
    Wi = np.imag(Wc).astype(np.float32)
    # [xy, t, i, o]
    Wr_x = Wr.transpose(3, 4, 0, 1, 2).reshape(MX * MY, T, U, U)
    Wi_x = Wi.transpose(3, 4, 0, 1, 2).reshape(MX * MY, T, U, U)
    # arr[xy, j, tt, i, c, m=(oh,c',o_lo)]
    arr = np.empty((MX * MY, 3, 2, U, 2, 2, 2, 16), np.float16)
    for j in range(3):
        for tt in range(2):
            t = j + 3 * tt
            wr = Wr_x[:, t].reshape(MX * MY, U, 2, 16)
            wi = Wi_x[:, t].reshape(MX * MY, U, 2, 16)
            arr[:, j, tt, :, 0, :, 0, :] = wr
            arr[:, j, tt, :, 0, :, 1, :] = wi
            arr[:, j, tt, :, 1, :, 0, :] = -wi
            arr[:, j, tt, :, 1, :, 1, :] = wr

    in_maps = []
    for c in range(NC):
        w2c = (arr[c * XY_PER_CORE:(c + 1) * XY_PER_CORE]
               .reshape(4, 32, 3, 128, 64).transpose(2, 0, 3, 1, 4)
               .reshape(3, 4, 128, 2048))
        in_maps.append({
            "xsh": np.ascontiguousarray(xf[c * PAIRS_PER_CORE:(c + 1) * PAIRS_PER_CORE]),
            "W2d": np.ascontiguousarray(w2c),
            "ATs": ATs,
            "B2T": B2T,
            "IDN": np.eye(128, dtype=np.float16),
            "CsT": CsT,
            "CT2r": CT2r,
            "CT2i": CT2i,
        })
    return in_maps


def _host_post(results):
    out = np.empty((B, 1, U, H, W), np.complex64)
    inv = np.float32(1.0 / OUT_DESCALE)
    for c in range(NC):
        # raw layout [chl, hc, p, comp, w]; h = hc*128 + p
        arr = (np.asarray(results[c]["outp"])
               .reshape(CH_PER_CORE, 2, 128, 2, 256).astype(np.float32))
        arr = arr.transpose(0, 3, 1, 2, 4).reshape(CH_PER_CORE, 2, H, W)
        carr = (arr[:, 0] + 1j * arr[:, 1]).astype(np.complex64)
        for j in range(CH_PER_CORE):
            ch = c * CH_PER_CORE + j
            out[ch // U, 0, ch % U] = carr[j] * inv
    return out


def kernel(**inputs):
    nc = _get_nc()
    in_maps = _host_prep(inputs["input"], inputs["R_w"], inputs["Ws_w"], inputs["Wt_w"])
    res = bass_utils.run_bass_kernel_spmd(nc, in_maps, core_ids=list(range(NC)))
    return _host_post(res.results)


# revision 42
# speedup vs baseline: 1.0702x; 1.0394x over previous
"""Trainium2 Bass kernel for AutoRegressiveAdaptiveSpectralConv2d.

reference:  f = fft2(x)[..., :32, :32]
            o = einsum('btixy,tioxy->btoxy', f, R_w) * Ws_w
            o = (o * Wt/sum(Wt)).sum(t)            -> [B,1,U,32,32]
            out = ifft2(o, s=(256,256))            -> [B,1,U,256,256] complex64

v3 structure (8 cores, single SPMD launch, fp16 data / fp32 PSUM):
  Phase 1 runs in 3 waves of one (b,t) pair (32 images) per core:
    step A: PQT[w, (P|Q,kx)] = x-block(stationary) @ ats   (4 mm/img, 64 cols)
    step B: f_ps[(c|s,ky), (img,m)] = B2(stationary) @ PQT (2 mm per 8 imgs)
    combine on gpsimd: f_re = cP + sQ, f_im = cQ - sP -> f_stage fp16
    per-wave AllToAll redistributes f (b,t)-shard -> kx-shard, overlapped
    with the next wave's compute.
  Phase 2 (xy-sharded channel mix) is PSUM-accumulated across the 3 waves:
    wave j supplies k-chunk j (t = {j, 3+j} x i x re/im = 128 rows).
  One small AllToAll moves o to (b,o)-sharding; phase 3 does the
  zero-padded iFFT with 2-channel-batched 512-col matmuls.
Weights pre-scaled by 2^22 on host; host divides output by 2^38.
"""
import sys
import numpy as np

sys.path.insert(0, "/opt/trn_rl_repo")

import concourse.bass as bass
import concourse.bacc as bacc
import concourse.mybir as mybir
import concourse.tile as tile
from concourse import bass_utils

B, T, U, H, W = 4, 6, 32, 256, 256
MX, MY = 32, 32
NC = 8
PAIRS_PER_CORE = (B * T) // NC          # 3 (= waves)
CH_PER_CORE = (B * U) // NC             # 16
XY_PER_CORE = (MX * MY) // NC           # 128
W_SCALE = float(2 ** 22)
OUT_DESCALE = float(2 ** 22) * float(H * W)

F16 = mybir.dt.float16
F32 = mybir.dt.float32

# Cross-partition-base operands are illegal (NCC_IBIR297), and only one
# input may read PSUM (NCC_IBVF027): use per-cs PSUM tiles at base 0 and
# evict the sin products to SBUF before the combines.
CROSS_BASE = False


def _ap(t, offset, dims):
    """AP on a pool tile with explicit [step, count] dims (tile-relative)."""
    return bass.AP(t[:].tensor, offset, dims)


def build_nc(timing=False, local_exchange=False):
    nc = bacc.Bacc("TRN2", target_bir_lowering=False, debug=False, num_devices=NC)

    def ext_in(name, shape, dt):
        if timing:
            return nc.dram_tensor(name, shape, dt)
        return nc.dram_tensor(name, shape, dt, kind="ExternalInput")

    xsh = ext_in("xsh", [PAIRS_PER_CORE, U, H, W], F16)
    W2d = ext_in("W2d", [3, 4, 128, 2048], F16)
    ATs = ext_in("ATs", [H, 64], F16)
    B2T = ext_in("B2T", [W, 64], F16)
    IDN = ext_in("IDN", [128, 128], F16)
    CsT = ext_in("CsT", [MX, 512], F16)
    CT2r = ext_in("CT2r", [128, 256], F16)
    CT2i = ext_in("CT2i", [128, 256], F16)
    # outp layout: [chl, hc, p, comp, w] with h = hc*128 + p
    if timing:
        outp = nc.dram_tensor("outp", [CH_PER_CORE, 2, 128, 2, 256], F16)
        dummy_in = nc.dram_tensor("dummy_in", [1, 64], F16, kind="ExternalInput")
        dummy_out = nc.dram_tensor("dummy_out", [1, 64], F16, kind="ExternalOutput")
    else:
        outp = nc.dram_tensor("outp", [CH_PER_CORE, 2, 128, 2, 256], F16,
                              kind="ExternalOutput")

    with tile.TileContext(nc) as tc:
        with (
            tc.tile_pool(name="dram", bufs=1, space="DRAM") as dram,
            tc.tile_pool(name="consts", bufs=1) as consts,
            tc.tile_pool(name="p1x", bufs=6) as p1x,
            tc.tile_pool(name="p1pq", bufs=2) as p1pq,
            tc.tile_pool(name="p1fs", bufs=2) as p1fs,
            tc.tile_pool(name="p2w", bufs=1) as p2w,
            tc.tile_pool(name="p2fk", bufs=2) as p2fk,
            tc.tile_pool(name="p2o", bufs=1) as p2osb,
            tc.tile_pool(name="p3sb", bufs=2) as p3sb,
        ):
            # DRAM exchange buffers: chunk layout per dst/src = [xy128, (i,c)64]
            f_in = [dram.tile([NC, XY_PER_CORE, 64], F16, tag=f"fin{j}",
                              name=f"f_in{j}") for j in range(3)]
            f_out = [dram.tile([NC, XY_PER_CORE, 64], F16, tag=f"fout{j}",
                               name=f"f_out{j}") for j in range(3)]
            o_in = dram.tile([NC, 32, XY_PER_CORE], F16)
            o_out = dram.tile([NC, 32, XY_PER_CORE], F16)

            ats = consts.tile([128, 128], F16)   # [h%128, (hc, m=(P|Q,kx))]
            b2 = consts.tile([128, 128], F16)    # [w%128, (wc, (c|s, ky))]
            idn = consts.tile([128, 128], F16)
            cst = consts.tile([MX, 512], F16)
            ct2r = consts.tile([128, 256], F16)
            ct2i = consts.tile([128, 256], F16)

            # SBUF consts (scalar queue, emitted first)
            nc.scalar.dma_start(
                _ap(ats, 0, [[128, 128], [64, 2], [1, 64]]),
                bass.AP(ATs, 0, [[64, 128], [8192, 2], [1, 64]]))
            nc.scalar.dma_start(
                _ap(b2, 0, [[128, 128], [64, 2], [1, 64]]),
                bass.AP(B2T, 0, [[64, 128], [8192, 2], [1, 64]]))
            nc.scalar.dma_start(idn[:], IDN[:])
            nc.scalar.dma_start(cst[:], CsT[:])
            nc.scalar.dma_start(ct2r[:], CT2r[:])
            nc.scalar.dma_start(ct2i[:], CT2i[:])

            w_all = p2w.tile([128, 3 * 8192], F16, tag="wsb")

            # phase-2 accumulator (lives across all 3 waves)
            p2ctx = tc.tile_pool(name="p2ps", bufs=1, space="PSUM")
            p2ps = p2ctx.__enter__()
            p1actx = tc.tile_pool(name="p1psa", bufs=2, space="PSUM")
            p1psa = p1actx.__enter__()
            p1bctx = tc.tile_pool(name="p1psb", bufs=2, space="PSUM")
            p1psb = p1bctx.__enter__()
            # full-bank tile; partitions 0-63 used. Explicitly zeroed, then
            # all phase-2 matmuls pure-accumulate (order-independent).
            o_ps = p2ps.tile([128, 512], F32, tag="ops")  # [(oh,c',o), (b,kyy,kxl)]
            nc.vector.memset(_ap(o_ps, 0, [[512, 64], [1, 512]]), 0.0)

            cpeng = [nc.scalar.copy, nc.vector.tensor_copy]
            fk_tiles = []

            # ---- all x loads up-front on sync so no later-waiting DMA
            # (fk_raw waits on a collective) head-of-line blocks them ----
            x_tiles = []
            for j in range(3):
                for half in range(2):
                    x_sb = p1x.tile([128, 8192], F16, tag="x")
                    # wave 2 rides scalar (idle until copies ramp ~25us)
                    eng = nc.scalar if j == 2 else nc.sync
                    eng.dma_start(
                        _ap(x_sb, 0, [[8192, 128], [512, 16], [256, 2], [1, 256]]),
                        bass.AP(xsh, (j * U + half * 16) * H * W,
                                [[256, 128], [65536, 16], [32768, 2], [1, 256]]))
                    x_tiles.append(x_sb)
            # W2 after x on the same queue: [128 krow, (j3, kxl4, kyy32, m64)]
            for j in range(3):
                nc.sync.dma_start(
                    _ap(w_all, j * 8192, [[24576, 128], [2048, 4], [1, 2048]]),
                    bass.AP(W2d, j * 4 * 128 * 2048,
                            [[2048, 128], [128 * 2048, 4], [1, 2048]]))

            def wave(j):
                x_halves = x_tiles[2 * j:2 * j + 2]
                # ---- step A: PQT[w, m] per image, 4-img PSUM banks ----
                pqt_sb = p1pq.tile([128, 4096], F16, tag="pqt_sb")
                nevict = 0
                for half in range(2):
                    x_sb = x_halves[half]
                    for bk in range(4):
                        pqt_ps = p1psa.tile([128, 512], F32, tag="pqt_ps")
                        for il in range(4):
                            for wc in range(2):
                                for hc in range(2):
                                    nc.tensor.matmul(
                                        _ap(pqt_ps, (il * 2 + wc) * 64,
                                            [[512, 128], [1, 64]]),
                                        _ap(x_sb, (bk * 4 + il) * 512 + hc * 256
                                            + wc * 128, [[8192, 128], [1, 128]]),
                                        _ap(ats, hc * 64, [[128, 128], [1, 64]]),
                                        start=(hc == 0), stop=(hc == 1))
                        cpeng[nevict % 2](
                            _ap(pqt_sb, (half * 16 + bk * 4) * 128,
                                [[4096, 128], [1, 512]]),
                            pqt_ps[:])
                        nevict += 1
                # ---- step B + combine: f_stage [ky, (kx, i, c)] fp16 ----
                f_stage = p1fs.tile([32, 2048], F16, tag="fstage")
                for grp in range(4):       # 8 images each
                    mov = lambda wc, moff, cnt=64: _ap(
                        pqt_sb, grp * 8 * 128 + wc * 64 + moff,
                        [[4096, 128], [128, 8], [1, cnt]])
                    if CROSS_BASE:
                        f_ps = p1psb.tile([128, 512], F32, tag="fps")
                        for wc in range(2):
                            nc.tensor.matmul(
                                _ap(f_ps, 0, [[512, 64], [1, 512]]),
                                _ap(b2, wc * 64, [[128, 128], [1, 64]]),
                                mov(wc, 0),
                                start=(wc == 0), stop=(wc == 1))
                        # evict whole [64,512] to SBUF; combines read SBUF
                        f_ev = p1fs.tile([64, 512], F16, tag="fev")
                        cpeng[grp % 2](f_ev[:], _ap(f_ps, 0,
                                                    [[512, 64], [1, 512]]))
                        srcs = {  # (partition base, col base)
                            "cP": _ap(f_ev, 0, [[512, 32], [64, 8], [1, 32]]),
                            "sQ": _ap(f_ev, 32 * 512 + 32,
                                      [[512, 32], [64, 8], [1, 32]]),
                            "cQ": _ap(f_ev, 32, [[512, 32], [64, 8], [1, 32]]),
                            "sP": _ap(f_ev, 32 * 512,
                                      [[512, 32], [64, 8], [1, 32]]),
                        }
                    else:
                        ps_c = p1psb.tile([128, 512], F32, tag="fps_c")
                        ps_s = p1psb.tile([128, 512], F32, tag="fps_s")
                        for wc in range(2):
                            for cs, pst in ((0, ps_c), (1, ps_s)):
                                nc.tensor.matmul(
                                    _ap(pst, 0, [[512, 32], [1, 512]]),
                                    _ap(b2, wc * 64 + cs * 32,
                                        [[128, 128], [1, 32]]),
                                    mov(wc, 0),
                                    start=(wc == 0), stop=(wc == 1))
                        # evict sin products to SBUF (aligned partitions)
                        f_ev = p1fs.tile([32, 512], F16, tag="fev")
                        cpeng[grp % 2](f_ev[:], _ap(ps_s, 0,
                                                    [[512, 32], [1, 512]]))
                        srcs = {
                            "cP": _ap(ps_c, 0, [[512, 32], [64, 8], [1, 32]]),
                            "sQ": _ap(f_ev, 32, [[512, 32], [64, 8], [1, 32]]),
                            "cQ": _ap(ps_c, 32, [[512, 32], [64, 8], [1, 32]]),
                            "sP": _ap(f_ev, 0, [[512, 32], [64, 8], [1, 32]]),
                        }
                    # f_re = cP + sQ ; f_im = cQ - sP  (dst col = kx*64+i*2+c)
                    nc.vector.scalar_tensor_tensor(
                        _ap(f_stage, grp * 16, [[2048, 32], [2, 8], [64, 32]]),
                        srcs["cP"], 1.0, srcs["sQ"],
                        mybir.AluOpType.mult, mybir.AluOpType.add)
                    nc.vector.scalar_tensor_tensor(
                        _ap(f_stage, grp * 16 + 1,
                            [[2048, 32], [2, 8], [64, 32]]),
                        srcs["cQ"], 1.0, srcs["sP"],
                        mybir.AluOpType.mult, mybir.AluOpType.subtract)
                # ---- store to f_in[j]: chunk[d] = [xy128, (i,c)64] ----
                nc.gpsimd.dma_start(
                    bass.AP(f_in[j][:].tensor, 0,
                            [[64, 32], [8192, 8], [2048, 4], [1, 64]]),
                    _ap(f_stage, 0, [[2048, 32], [256, 8], [64, 4], [1, 64]]))
                # ---- AllToAll wave j ----
                if local_exchange:
                    nc.sync.dma_start(f_out[j][:], f_in[j][:])
                else:
                    nc.gpsimd.collective_compute(
                        "AllToAll", mybir.AluOpType.bypass,
                        replica_groups=[list(range(NC))],
                        ins=[f_in[j].opt()], outs=[f_out[j].opt()])
                # ---- fk load: [128 krow=(tt,i,c), (b4, xy128)] ----
                # straight load [xy, (b,tt,ic)] then 4 XBAR transposes
                # -> fk [128 = (tt,i,c), (b4, xy128)]
                fk_raw = p2fk.tile([128, 512], F16, tag="fk_raw")
                nc.sync.dma_start(
                    _ap(fk_raw, 0, [[512, 128], [64, 8], [1, 64]]),
                    bass.AP(f_out[j][:].tensor, 0,
                            [[64, 128], [8192, 8], [1, 64]]))
                fk = p2fk.tile([128, 512], F16, tag="fk")
                for bp in range(2):
                    tr_ps = p1psa.tile([128, 256], F16, tag="tr", bufs=1)
                    for bh in range(2):
                        b = bp * 2 + bh
                        nc.tensor.transpose(
                            _ap(tr_ps, bh * 128, [[256, 128], [1, 128]]),
                            _ap(fk_raw, b * 128, [[512, 128], [1, 128]]),
                            idn[:])
                    cpeng[bp % 2](_ap(fk, bp * 256, [[512, 128], [1, 256]]),
                                  tr_ps[:])
                fk_tiles.append(fk)

            def p2_chunk(j):
                # o_ps cols = (b4, kyy32, kxl4)
                fk = fk_tiles[j]
                for kxl in range(4):
                    for kyy in range(32):
                        nc.tensor.matmul(
                            _ap(o_ps, kyy * 4 + kxl, [[512, 64], [128, 4]]),
                            _ap(w_all, j * 8192 + kxl * 2048 + kyy * 64,
                                [[24576, 128], [1, 64]]),
                            _ap(fk, kxl * 32 + kyy, [[512, 128], [128, 4]]),
                            start=False, stop=False, skip_group_check=True)

            # ---- schedule: w0, w1, p2c0, w2, p2c1, p2c2 ----
            wave(0)
            wave(1)
            p2_chunk(0)
            wave(2)
            p2_chunk(1)
            p2_chunk(2)

            # ---- o evict + exchange ----
            # chunk layout per dst: [32 ky, 128 = (c,o_lo)*4 + kxl]
            o_sb = p2osb.tile([64, 512], F16, tag="osb")
            nc.vector.tensor_copy(o_sb[:], _ap(o_ps, 0, [[512, 64], [1, 512]]))
            for d in range(NC):
                eng = (nc.sync, nc.scalar, nc.gpsimd)[d % 3]
                eng.dma_start(
                    bass.AP(o_in[:].tensor, d * 4096,
                            [[4, 32], [128, 32], [1, 4]]),
                    _ap(o_sb, (d % 2) * 32 * 512 + (d // 2) * 128,
                        [[512, 32], [4, 32], [1, 4]]))
            p1bctx.__exit__(None, None, None)
            p1actx.__exit__(None, None, None)
            p2ctx.__exit__(None, None, None)
            if local_exchange:
                nc.sync.dma_start(o_out[:], o_in[:])
            else:
                nc.gpsimd.collective_compute(
                    "AllToAll", mybir.AluOpType.bypass,
                    replica_groups=[list(range(NC))],
                    ins=[o_in.opt()], outs=[o_out.opt()])

            # ---- phase 3: zero-padded iFFT, 2 channels per group ----
            # straight load then same-partition col rearrange
            ot_raw = consts.tile([MX, CH_PER_CORE * 64], F16)  # [ky,(s,p,kxl)]
            nc.sync.dma_start(
                _ap(ot_raw, 0, [[1024, 32], [128, 8], [1, 128]]),
                bass.AP(o_out[:].tensor, 0, [[128, 32], [4096, 8], [1, 128]]))
            ot = consts.tile([MX, CH_PER_CORE * 64], F16)  # [ky,(ch,c,kx)]
            for s in range(NC):
                eng = (nc.vector.tensor_copy, nc.gpsimd.tensor_copy,
                       nc.scalar.copy)[s % 3]
                eng(_ap(ot, s * 4, [[1024, 32], [64, 16], [32, 2], [1, 4]]),
                    _ap(ot_raw, s * 128, [[1024, 32], [4, 16], [64, 2], [1, 4]]))
            p3actx = tc.tile_pool(name="p3psa", bufs=2, space="PSUM")
            p3psa = p3actx.__enter__()
            p3bctx = tc.tile_pool(name="p3psb", bufs=3, space="PSUM")
            p3psb = p3bctx.__enter__()
            def g_group(gi):
                g2 = p3sb.tile([128, 512], F16, tag="g2")
                for cl in range(2):
                    chl = gi * 2 + cl
                    g_ps = p3psa.tile([128, 256], F32, tag="g_ps")
                    for cs in range(2):
                        nc.tensor.matmul(
                            _ap(g_ps, cs * 64 * 256, [[256, 64], [1, 256]]),
                            _ap(ot, chl * 64, [[1024, 32], [1, 64]]),
                            _ap(cst, cs * 256, [[512, 32], [1, 256]]),
                            start=True, stop=True)
                    cpeng[cl](_ap(g2, cl * 256, [[512, 128], [1, 256]]),
                              g_ps[:])
                return g2

            def p_group(gi, g2):
                # per-hc double-bank PSUM tile; both comps evicted in ONE
                # copy (fewer instructions on the 2 PSUM-capable lanes)
                for hc in range(2):
                    p_ps2 = p3psb.tile([128, 1024], F32, tag="p_ps", bufs=3)
                    for comp in range(2):
                        nc.tensor.matmul(
                            _ap(p_ps2, comp * 512, [[1024, 128], [1, 512]]),
                            _ap(ct2r if comp == 0 else ct2i, hc * 128,
                                [[256, 128], [1, 128]]),
                            g2[:], start=True, stop=True)
                    oint = p3sb.tile([128, 1024], F16, tag=f"oint{hc}")
                    # src cols (comp,ch,w) -> dst cols (ch,comp,w)
                    cpeng[(2 * gi + hc) % 2](
                        _ap(oint, 0, [[1024, 128], [512, 2], [256, 2], [1, 256]]),
                        _ap(p_ps2, 0, [[1024, 128], [256, 2], [512, 2], [1, 256]]))
                    for cl in range(2):
                        chl = gi * 2 + cl
                        eng = (nc.sync, nc.gpsimd)[(2 * gi + cl + hc) % 2]
                        eng.dma_start(
                            bass.AP(outp, chl * 2 * H * W + hc * 65536,
                                    [[512, 128], [1, 512]]),
                            _ap(oint, cl * 512, [[1024, 128], [1, 512]]))

            # software pipeline: g one group ahead so the PE never waits
            # on the g-evict copies before each p-step
            prev = None
            for gi in range(CH_PER_CORE // 2):
                g2 = g_group(gi)
                if prev is not None:
                    p_group(*prev)
                prev = (gi, g2)
            p_group(*prev)
            p3bctx.__exit__(None, None, None)
            p3actx.__exit__(None, None, None)
            if timing:
                nc.sync.dma_start(bass.AP(dummy_out, 0, [[64, 1], [1, 64]]),
                                  bass.AP(dummy_in, 0, [[64, 1], [1, 64]]))
    nc.compile()
    return nc


_NC_CACHE = None


def _get_nc():
    global _NC_CACHE
    if _NC_CACHE is None:
        _NC_CACHE = build_nc()
    return _NC_CACHE


def _host_prep(x, R_w, Ws_w, Wt_w):
    x = np.asarray(x)
    R_w = np.asarray(R_w)
    Ws_w = np.asarray(Ws_w, dtype=np.float32)
    Wt_w = np.asarray(Wt_w, dtype=np.float32)

    xf = x.reshape(B * T, U, H, W).astype(np.float16)

    h = np.arange(H)[:, None]
    k = np.arange(MX)[None, :]
    ang = 2.0 * np.pi * h * k / H
    ATs = np.concatenate([np.cos(ang), -np.sin(ang)], axis=1).astype(np.float16)

    w = np.arange(W)[:, None]
    ky = np.arange(MY)[None, :]
    angb = 2.0 * np.pi * w * ky / W
    B2T = np.concatenate([np.cos(angb), np.sin(angb)], axis=1).astype(np.float16)

    xg = np.arange(MX)[:, None]
    wg = np.arange(W)[None, :]
    ang2 = 2.0 * np.pi * xg * wg / W
    cos2 = np.cos(ang2).astype(np.float32)
    sin2 = np.sin(ang2).astype(np.float32)
    CsT = np.concatenate([cos2, sin2], axis=1).astype(np.float16)
    CT2r = np.concatenate([cos2, -sin2, -sin2, -cos2], axis=0).astype(np.float16)
    CT2i = np.concatenate([sin2, cos2, cos2, -sin2], axis=0).astype(np.float16)

    wt = (Wt_w / Wt_w.sum()).reshape(T)
    Wc = (R_w * Ws_w[None, None, None]
          * wt[:, None, None, None, None].astype(np.float32) * W_SCALE)
    Wr = np.real(Wc).astype(np.float32)
    Wi = np.imag(Wc).astype(np.float32)
    # [xy, t, i, o]
    Wr_x = Wr.transpose(3, 4, 0, 1, 2).reshape(MX * MY, T, U, U)
    Wi_x = Wi.transpose(3, 4, 0, 1, 2).reshape(MX * MY, T, U, U)
    # arr[xy, j, tt, i, c, m=(oh,c',o_lo)]
    arr = np.empty((MX * MY, 3, 2, U, 2, 2, 2, 16), np.float16)
    for j in range(3):
        for tt in range(2):
            t = j + 3 * tt
            wr = Wr_x[:, t].reshape(MX * MY, U, 2, 16)
            wi = Wi_x[:, t].reshape(MX * MY, U, 2, 16)
            arr[:, j, tt, :, 0, :, 0, :] = wr
            arr[:, j, tt, :, 0, :, 1, :] = wi
            arr[:, j, tt, :, 1, :, 0, :] = -wi
            arr[:, j, tt, :, 1, :, 1, :] = wr

    in_maps = []
    for c in range(NC):
        w2c = (arr[c * XY_PER_CORE:(c + 1) * XY_PER_CORE]
               .reshape(4, 32, 3, 128, 64).transpose(2, 0, 3, 1, 4)
               .reshape(3, 4, 128, 2048))
        in_maps.append({
            "xsh": np.ascontiguousarray(xf[c * PAIRS_PER_CORE:(c + 1) * PAIRS_PER_CORE]),
            "W2d": np.ascontiguousarray(w2c),
            "ATs": ATs,
            "B2T": B2T,
            "IDN": np.eye(128, dtype=np.float16),
            "CsT": CsT,
            "CT2r": CT2r,
            "CT2i": CT2i,
        })
    return in_maps


def _host_post(results):
    out = np.empty((B, 1, U, H, W), np.complex64)
    inv = np.float32(1.0 / OUT_DESCALE)
    for c in range(NC):
        # raw layout [chl, hc, p, comp, w]; h = hc*128 + p
        arr = (np.asarray(results[c]["outp"])
               .reshape(CH_PER_CORE, 2, 128, 2, 256).astype(np.float32))
        arr = arr.transpose(0, 3, 1, 2, 4).reshape(CH_PER_CORE, 2, H, W)
        carr = (arr[:, 0] + 1j * arr[:, 1]).astype(np.complex64)
        for j in range(CH_PER_CORE):
            ch = c * CH_PER_CORE + j
            out[ch // U, 0, ch % U] = carr[j] * inv
    return out


def kernel(**inputs):
    nc = _get_nc()
    in_maps = _host_prep(inputs["input"], inputs["R_w"], inputs["Ws_w"], inputs["Wt_w"])
    res = bass_utils.run_bass_kernel_spmd(nc, in_maps, core_ids=list(range(NC)))
    return _host_post(res.results)
